# Initial kernel scaffold
#
"""Channel-attention kernel for Trainium2 (8 NeuronCores, batch-parallel).

Reference computation per batch b (feat (C, HW2), word_emb (N, D)):
    we0   = word_emb @ W_fc^T                 (N, HW2)
    S     = feat @ we0^T                      (C, N)   [b_fc shifts every logit
                                                        of a row equally -> the
                                                        softmax is invariant]
    A     = softmax(S, axis=-1)
    out   = A @ we0 + b_fc                    (C, HW2) [b_fc added on host]

Precision scheme: fp16 hi/lo pairs everywhere (hi = fp16(x), lo = fp16(x-hi);
3-chain products hi*hi + hi*lo + lo*hi carry ~22 mantissa bits, needed because
softmax logits (sigma ~ 32) demand small ABSOLUTE error). A single-pass f32r
matmul was measured at ~2e-2 max logit error on HW (TF32-like) -- not enough.

Host marshalling:
  - feat pre-transposed to (HW2, C) and split hi/lo, interleaved per row as
    [hi(512) | lo(512)] -> 2KB DMA lines; contraction dim on SBUF partitions.
  - W_fc^T and word_emb^T pre-transposed + hi/lo split on host (kills all
    device-side setup transposes and their weight loads).
  - output is fp16 (halves output DMA); b_fc added on host in fp32.

Device dataflow per batch (one NeuronCore handles B/8 = 4 batches):
    we0 hi/lo   = 3 fp16 chains wembT^T @ wfcT   (PE, 4 LDW + 12 MM F=512)
    wt hi/lo    = PE transposes of we0 hi/lo     (16 transposes)
    S^T         = 3 fp16 chains wt^T @ FT        (PE, 24 MM F=512)
    eT          = exp(0.5 S^T - 48)^2 as f32r    (ACT exp + DVE square; fixed
                                                  -96 shift is softmax-exact)
    sums        = ones^T @ eT                    (PE f32r, 1 MM F=512)
    rb          = bcast 1/sums via K=1 f32r MM   (PE, 1 MM F=512; f32r runs
                                                  1 cyc/row at F>=512 vs 4 for
                                                  fp32 -- old fp32 broadcasts
                                                  measured 5.9us total)
    at16        = fp16(eT * rb)                  (DVE)
    O           = at16-chunk^T @ we0hi           (PE, 8 MM F=512), fp16 out

GPSIMD cross-lane ops are avoided entirely: a partition-axis max/reduce
measured ~40us per [77,512] call on HW.

Emission order pipelines three batches so the in-order PE queue never heads
on a non-PE softmax chain: softA(b-1) | prepB(b) | softB(b-1) | prepA(b+1) |
score(b) | o(b-1); prepA(b+1) covers score(b)'s wait on the wt DVE copies
(needs med pool bufs=3 -- with 2, b+1's we0 split aliases b-1's live operands
and serializes). Keeps the PE stream dense for the HAM clock gate (cold PE
runs at 1.2 GHz; LDWEIGHTS is never hidden here -- enable-ldw-opt=false --
so per-MM cost is stream + ~107ns weight load, making MM count the currency).
"""

import numpy as np

import concourse.bass as bass
import concourse.mybir as mybir
import concourse.tile as tile
from concourse import bacc
from concourse.bass import ds, ts
from concourse.bass_utils import run_bass_kernel_spmd
from concourse.masks import make_identity

B, C, HW2 = 32, 512, 1024
N_WORDS, WORD_DIM = 77, 256
H = W = 32
N_CORES = 8
BPC = B // N_CORES  # batches per core
NP = 80  # n padded to even (f32r/ISA friendliness + zero-padded stationaries)

FP32 = mybir.dt.float32
FP16 = mybir.dt.float16
AF = mybir.ActivationFunctionType

LAST_RESULT = None  # BassKernelResults of the most recent run (for test.py)


def _body(nc, tc, ftp_d, wfcT_d, out_d):
    from contextlib import ExitStack

    with ExitStack() as ctx:
        const = ctx.enter_context(tc.tile_pool(name="const", bufs=1))
        big = ctx.enter_context(tc.tile_pool(name="big", bufs=3))
        med = ctx.enter_context(tc.tile_pool(name="med", bufs=3))
        outp = ctx.enter_context(tc.tile_pool(name="outp", bufs=4))
        mm_ps = ctx.enter_context(tc.tile_pool(name="mm_ps", bufs=3, space="PSUM"))
        sp_ps = ctx.enter_context(tc.tile_pool(name="sp_ps", bufs=2, space="PSUM"))
        tp_ps = ctx.enter_context(tc.tile_pool(name="tp_ps", bufs=2, space="PSUM"))
        su_ps = ctx.enter_context(tc.tile_pool(name="su_ps", bufs=1, space="PSUM"))

        identh = const.tile([128, 128], FP16)
        ident = const.tile([128, 128], FP32)
        make_identity(nc, ident[:])
        nc.vector.tensor_copy(identh[:], ident[:])
        ones_f = const.tile([128, 8], FP32)
        nc.gpsimd.memset(ones_f[:], 1.0)
        onesr = const.tile([128, 8], mybir.dt.float32r)
        nc.vector.tensor_copy(onesr[:], ones_f[:])
        ones1f = const.tile([128, 128], FP32)
        nc.gpsimd.memset(ones1f[:], 1.0)
        ones1r = const.tile([128, 128], mybir.dt.float32r)
        nc.vector.tensor_copy(ones1r[:], ones1f[:])
        ebias = const.tile([128, 1], FP32)
        nc.gpsimd.memset(ebias[:], -48.0)

        # W_fc^T fp16 hi/lo + all 4 batches' word_emb^T piggybacked as tail
        # columns (one single-descriptor DMA; per-batch wembT DMAs were 256x
        # 320B descriptor storms: ~5us to issue + ~10us to execute each).
        # Row (p, dc): [wfc hi(1024) | wfc lo(1024) | b0 hi80|lo80 | b1 ... ]
        wfcT = const.tile([128, 2, 2048 + BPC * 2 * NP], FP16)
        nc.sync.dma_start(wfcT[:], wfcT_d.rearrange("(t p) x -> p t x", p=128))

        def load(b):
            st = {}
            ft = st["ft"] = big.tile([128, 8, 1024], FP16, tag="ft", name="ft")
            ft_src = ftp_d[b].rearrange("(t p) x -> p t x", p=128)
            for kt in range(8):
                nc.sync.dma_start(ft[:, kt, :], ft_src[:, kt, :])
            st["b"] = b
            return st

        def prep_a(st):
            # we0 = word_emb @ W_fc^T (77, 1024): 3 fp16 chains, 4 LDW
            wb = 2048 + st["b"] * 2 * NP
            ph = [
                mm_ps.tile([128, 512], FP32, tag="mm", name=f"we0p{h}")
                for h in range(2)
            ]
            n_in = {0: 0, 1: 0}
            for dc in range(2):
                for sl in (ds(0, NP), ds(NP, NP)):  # hi stationary, then lo
                    hi_st = sl.start == 0
                    for h in range(2):
                        for w_base in (0, 1024):
                            w_hi = w_base == 0
                            if not hi_st and not w_hi:
                                continue  # lo@lo dropped
                            nc.tensor.matmul(
                                ph[h][:NP, :],
                                wfcT[:, dc, ds(wb + sl.start, NP)],
                                wfcT[:, dc, ds(w_base + h * 512, 512)],
                                start=(n_in[h] == 0),
                                stop=(n_in[h] == 5),
                            )
                            n_in[h] += 1
            # fp16 split of we0 straight from PSUM
            we0hi = st["we0h"] = med.tile([128, 1024], FP16, tag="we0hi", name="we0hi")
            we0lo = st["we0l"] = med.tile([128, 1024], FP16, tag="we0lo", name="we0lo")
            for h in range(2):
                nc.vector.tensor_copy(we0hi[:NP, ds(h * 512, 512)], ph[h][:NP, :])
            for h in range(2):
                nc.vector.tensor_sub(
                    we0lo[:NP, ds(h * 512, 512)],
                    ph[h][:NP, :],
                    we0hi[:NP, ds(h * 512, 512)],
                )
            return st

        def prep_b(st):
            # wt hi/lo = we0 hi/lo transposed: 16 PE transposes -> psum fp16
            wthi = st["wthi"] = med.tile([128, 8, NP], FP16, tag="wthi", name="wthi")
            wtlo = st["wtlo"] = med.tile([128, 8, NP], FP16, tag="wtlo", name="wtlo")
            nc.gpsimd.memset(wthi[:, :, 77:], 0.0)
            nc.gpsimd.memset(wtlo[:, :, 77:], 0.0)
            for src, dst in ((st["we0h"], wthi), (st["we0l"], wtlo)):
                pt = tp_ps.tile([128, 8, NP], FP16, tag="tp")
                for g in range(8):
                    nc.tensor.matmul(
                        pt[:, g, :77],
                        src[:77, ts(g, 128)],
                        identh[:77, :77],
                        is_transpose=True,
                        start=(g == 0),
                        stop=(g == 7),
                    )
                nc.vector.tensor_copy(dst[:, :, :77], pt[:, :, :77])
            return st

        def score(st):
            # S^T = wt^T @ FT (77, 512): 3 fp16 chains, 16 LDW + 24 MM
            ft, wthi, wtlo = st["ft"], st["wthi"], st["wtlo"]
            sps = st["sps"] = sp_ps.tile([128, 512], FP32, tag="sp", name="sps")
            i_mm = 0
            for kt in range(8):
                for lhs, sl in (
                    (wthi, ds(0, 512)),  # hi @ hi
                    (wthi, ds(512, 512)),  # hi @ lo (same stationary)
                    (wtlo, ds(0, 512)),  # lo @ hi
                ):
                    nc.tensor.matmul(
                        sps[:NP, :],
                        wthi[:, kt, :] if lhs is wthi else wtlo[:, kt, :],
                        ft[:, kt, sl],
                        start=(i_mm == 0),
                        stop=(i_mm == 23),
                    )
                    i_mm += 1

        def soft_a(st):
            # E = exp(S - 96) via exp(0.5 S - 48)^2 (fixed shift: softmax-
            # exact, overflow-safe); eT in f32r for the PE sums matmul
            sps = st["sps"]
            ehalf = med.tile([128, 512], FP32, tag="ehalf")
            nc.scalar.activation(
                ehalf[:77, :], sps[:77, :], AF.Exp, bias=ebias[:77, :], scale=0.5
            )
            eT = st["eT"] = med.tile(
                [128, 512], mybir.dt.float32r, tag="eT", name="eT"
            )
            nc.vector.tensor_mul(eT[:77, :], ehalf[:77, :], ehalf[:77, :])

        def soft_b(st):
            # denominators: (1, C) row via f32r ones matmul, 1/row, f32r
            # broadcast back to 80 rows, A = E/sums -> fp16
            eT = st["eT"]
            sus = su_ps.tile([128, 512], FP32, tag="sums")
            nc.tensor.matmul(sus[:8, :], onesr[:77, :], eT[:77, :])
            rrow = med.tile([128, 512], FP32, tag="rrow")
            nc.vector.reciprocal_approx_fast(rrow[:1, :], sus[:1, :])
            rrowr = med.tile([128, 512], mybir.dt.float32r, tag="rrowr")
            nc.vector.tensor_copy(rrowr[:1, :], rrow[:1, :])
            rb = mm_ps.tile([128, 512], FP32, tag="mm", name="rb")
            nc.tensor.matmul(rb[:NP, :], ones1r[:1, :NP], rrowr[:1, :])
            at16 = st["at"] = med.tile([128, 512], FP16, tag="at", name="at")
            nc.vector.tensor_mul(at16[:77, :], eT[:77, :], rb[:77, :])

        def o_phase(st, b):
            # per c-tile: O_un = A-slice^T @ we0hi, sums via ones(F=2) on the
            # same stationary; normalize during PSUM->SBUF copy; fp16 out
            at16, we0hi = st["at"], st["we0h"]
            for ct in range(4):
                po0 = mm_ps.tile([128, 512], FP32, tag="mm")
                nc.tensor.matmul(po0[:], at16[:77, ts(ct, 128)], we0hi[:77, :512])
                po1 = mm_ps.tile([128, 512], FP32, tag="mm")
                nc.tensor.matmul(po1[:], at16[:77, ts(ct, 128)], we0hi[:77, 512:1024])
                ob = outp.tile([128, 1024], FP16, tag="outb")
                nc.scalar.copy(ob[:, :512], po0[:])
                nc.vector.tensor_copy(ob[:, 512:], po1[:])
                nc.sync.dma_start(out_d[b, ts(ct, 128), :], ob[:])

        # ---- software pipeline ----
        states = {}
        states[0] = load(0)
        prep_a(states[0])
        prep_b(states[0])
        states[1] = load(1)
        score(states[0])
        prep_a(states[1])
        states[2] = load(2)
        for b in range(1, BPC):
            soft_a(states[b - 1])
            prep_b(states[b])
            soft_b(states[b - 1])
            if b + 1 < BPC:
                prep_a(states[b + 1])
            score(states[b])
            o_phase(states[b - 1], b - 1)
            if b + 2 < BPC:
                states[b + 2] = load(b + 2)
            del states[b - 1]
        soft_a(states[BPC - 1])
        soft_b(states[BPC - 1])
        o_phase(states[BPC - 1], BPC - 1)


def _build():
    nc = bacc.Bacc(
        "TRN2",
        target_bir_lowering=False,
        debug=False,
        enable_asserts=False,
        num_devices=N_CORES,
    )
    ftp_d = nc.declare_dram_parameter("ftp", [BPC, HW2, 2 * C], FP16, isOutput=False)
    wfcT_d = nc.declare_dram_parameter(
        "wfcT", [WORD_DIM, 2 * HW2 + BPC * 2 * NP], FP16, isOutput=False
    )
    out_d = nc.declare_dram_parameter("out", [BPC, C, HW2], FP16, isOutput=True)
    with tile.TileContext(nc) as tc:
        _body(nc, tc, ftp_d, wfcT_d, out_d)
    nc.finalize()
    return nc


_CACHE = {}


def kernel(feat, word_emb, W_fc, b_fc, **run_kwargs):
    global LAST_RESULT
    feat = np.asarray(feat, dtype=np.float32).reshape(B, C, HW2)
    word_emb = np.ascontiguousarray(np.asarray(word_emb, dtype=np.float32))
    W_fc = np.ascontiguousarray(np.asarray(W_fc, dtype=np.float32))
    b_fc = np.asarray(b_fc, dtype=np.float32)

    # feat -> (B, HW2, C) fp16 hi/lo interleaved per row [hi(512) | lo(512)]
    featT = np.ascontiguousarray(feat.transpose(0, 2, 1))
    fthi = featT.astype(np.float16)
    ftlo = (featT - fthi.astype(np.float32)).astype(np.float16)
    ftp = np.empty((B, HW2, 2 * C), dtype=np.float16)
    ftp[:, :, :C] = fthi
    ftp[:, :, C:] = ftlo

    # word_emb^T -> (B, WORD_DIM, 2*NP) fp16 [hi(80) | lo(80)], cols 77:80 = 0
    wembT = np.ascontiguousarray(word_emb.transpose(0, 2, 1))  # (B, 256, 77)
    wehi = wembT.astype(np.float16)
    welo = (wembT - wehi.astype(np.float32)).astype(np.float16)
    wembTp = np.zeros((B, WORD_DIM, 2 * NP), dtype=np.float16)
    wembTp[:, :, :77] = wehi
    wembTp[:, :, NP : NP + 77] = welo

    # W_fc^T -> (256, 2*HW2) fp16 [hi(1024) | lo(1024)]
    wfcT = np.ascontiguousarray(W_fc.T)  # (256, 1024)
    wfhi = wfcT.astype(np.float16)
    wflo = (wfcT - wfhi.astype(np.float32)).astype(np.float16)
    wfcTp = np.zeros((N_CORES, WORD_DIM, 2 * HW2 + BPC * 2 * NP), dtype=np.float16)
    wfcTp[:, :, :HW2] = wfhi
    wfcTp[:, :, HW2 : 2 * HW2] = wflo
    for i in range(N_CORES):
        for b in range(BPC):
            wfcTp[i, :, 2 * HW2 + b * 2 * NP : 2 * HW2 + (b + 1) * 2 * NP] = wembTp[
                i * BPC + b
            ]

    if "nc" not in _CACHE:
        _CACHE["nc"] = _build()
    nc = _CACHE["nc"]

    in_maps = [
        {
            "ftp": ftp[i * BPC : (i + 1) * BPC],
            "wfcT": wfcTp[i],
        }
        for i in range(N_CORES)
    ]
    res = run_bass_kernel_spmd(nc, in_maps, list(range(N_CORES)), **run_kwargs)
    LAST_RESULT = res
    out16 = np.concatenate([res.results[i]["out"] for i in range(N_CORES)], axis=0)
    # b_fc shifts all logits of a softmax row equally (no effect on A) and
    # adds linearly to the output: out = A @ we0 + b_fc. Exact identity.
    out = out16.astype(np.float32) + b_fc.reshape(1, 1, HW2)
    return out.reshape(B, C, H, W).astype(np.float32)



# revision 10
# speedup vs baseline: 1.2458x; 1.2458x over previous
"""Channel-attention kernel for Trainium2 (8 NeuronCores, batch-parallel).

Reference computation per batch b (feat (C, HW2), word_emb (N, D)):
    we0   = word_emb @ W_fc^T                 (N, HW2)
    S     = feat @ we0^T                      (C, N)   [b_fc shifts every logit
                                                        of a row equally -> the
                                                        softmax is invariant]
    A     = softmax(S, axis=-1)
    out   = A @ we0 + b_fc                    (C, HW2) [b_fc added on host]

Precision scheme: fp16 2-chain with an fp8 lo-correction for feat. The
harness gate is scale-relative absmax (~2e-2); this scheme measures 9.3e-3
on HW. Structure:
    S ~= fp16(we0) @ feat_hi  +  [fp16(we0)*2^-10]_e5m2 @ [feat_lo*2^10]_e5m2
The fp8 product scales cancel exactly, so the correction accumulates into
the same PSUM group; feat_lo at fp8 cuts its HBM traffic in half.

Device dataflow per batch (one NeuronCore handles B/8 = 4 batches):
    we0hi    = fp16(wembT_hi^T @ (wfc_hi|wfc_lo))   (PE, 4 LDW + 8 MM F=512)
    wt       = PE transposes of we0hi               (8 transposes)
    wt8      = e5m2(wt * 2^-10)                     (ACT scaled copy)
    S^T      = wt^T@FT_hi + wt8^T@FT_lo8            (PE, 8 fp16 MM + 4
                                                     DoubleRow fp8 MM F=512)
    eT       = exp(0.5 S^T - 48)^2 as f32r          (ACT exp + DVE square)
    sums     = ones^T @ eT                          (PE f32r, 1 MM F=512)
    rb       = bcast 1/sums via K=1 f32r MM         (PE, 1 MM F=512)
    at16     = fp16(eT * rb)                        (DVE)
    O        = at16-chunk^T @ we0hi                 (PE, 8 MM F=512), fp16 out

DMA plumbing (from trace analysis):
  - The 16 DMA engines service outstanding descriptors ROUND-ROBIN, so
    issue order does not give delivery order: every concurrently-queued
    descriptor steals bandwidth from the one you need first. Inputs are
    therefore gated into an explicitly SEQUENTIAL chain with 1-element
    "corner copy" dependencies on the (otherwise idle) GpSimd queue:
    wfcT -> wflo -> ft16(0) -> ft16(1) -> ... and a parallel fp8 chain
    ft8(0) -> ft8(1) -> ... headed by one DVE cast off ft16(0).
  - All tensors are host-pre-swizzled so each DMA is one descriptor with
    2-8KB contiguous per-partition lines (512B fp8 lines measured ~18%
    slower aggregate).
  - Output DMAs stay on the Sync queue and trail the input chain.

Compute scheduling: slot b emission = prep_b(b) | soft_b(b-1) | prep_a(b+1)
| score(b) | soft_a(b) | o(b-1).  Queue-order invariants: exp(b) lands on
the ACT queue BEFORE o(b-1)'s PSUM->SBUF copies, recip(b-1) lands on the
DVE queue before o(b-1)'s copies (strict-FIFO queues otherwise head-of-line
block the softmax chain behind output copies, idling the PE until the HAM
clock-gate re-throttles it to 1.2 GHz).  o(BPC-2)'s copies go ACT-only so
the epilogue's recip/at16 are not queued behind them on the DVE.
"""

import numpy as np
import ml_dtypes

import concourse.bass as bass
import concourse.mybir as mybir
import concourse.tile as tile
from concourse import bacc
from concourse.bass import ds, ts
from concourse.bass_utils import run_bass_kernel_spmd
from concourse.masks import make_identity

B, C, HW2 = 32, 512, 1024
N_WORDS, WORD_DIM = 77, 256
H = W = 32
N_CORES = 8
BPC = B // N_CORES  # batches per core
NP = 80  # n padded to even (f32r/ISA friendliness + zero-padded stationaries)

FP32 = mybir.dt.float32
FP16 = mybir.dt.float16
FP8 = mybir.dt.float8e5
F32R = mybir.dt.float32r
AF = mybir.ActivationFunctionType

TAIL0 = HW2  # wembT-hi columns start within wfcT (per dc)
WHI_W = HW2 + BPC * NP  # per-dc row of wfcT: [wfc_hi | wembT_hi*4]
LOSCALE = 2.0**10
USE_DR = True  # DoubleRow fp8 for the score lo-correction (4 MMs vs 8)

LAST_RESULT = None  # BassKernelResults of the most recent run (for test.py)


def _body(nc, tc, ftp16_d, ftp8_d, wfcT_d, wflo_d, out_d):
    from contextlib import ExitStack

    with ExitStack() as ctx:
        const = ctx.enter_context(tc.tile_pool(name="const", bufs=1))
        big = ctx.enter_context(tc.tile_pool(name="big", bufs=4))
        med = ctx.enter_context(tc.tile_pool(name="med", bufs=3))
        outp = ctx.enter_context(tc.tile_pool(name="outp", bufs=8))
        mm_ps = ctx.enter_context(tc.tile_pool(name="mm_ps", bufs=4, space="PSUM"))
        sp_ps = ctx.enter_context(tc.tile_pool(name="sp_ps", bufs=2, space="PSUM"))
        tp_ps = ctx.enter_context(tc.tile_pool(name="tp_ps", bufs=1, space="PSUM"))
        su_ps = ctx.enter_context(tc.tile_pool(name="su_ps", bufs=1, space="PSUM"))

        wfcT = const.tile([128, 2, WHI_W], FP16)
        nc.sync.dma_start(wfcT[:], wfcT_d.rearrange("p (t x) -> p t x", t=2))
        wflo = const.tile([128, 2, HW2], FP16)

        # constants (also fills the GpSimd queue before the DMA gate chain)
        identh = const.tile([128, 128], FP16)
        ident = const.tile([128, 128], FP32)
        make_identity(nc, ident[:])
        nc.vector.tensor_copy(identh[:], ident[:])
        ones_f = const.tile([128, 8], FP32)
        nc.gpsimd.memset(ones_f[:], 1.0)
        onesr = const.tile([128, 8], F32R)
        nc.vector.tensor_copy(onesr[:], ones_f[:])
        ones1f = const.tile([128, 128], FP32)
        nc.gpsimd.memset(ones1f[:], 1.0)
        ones1r = const.tile([128, 128], F32R)
        nc.vector.tensor_copy(ones1r[:], ones1f[:])
        ebias = const.tile([128, 1], FP32)
        nc.gpsimd.memset(ebias[:], -48.0)
        # wt double-buffer lives in const so its pad columns are zeroed once
        wthi_c = const.tile([128, 2, 8, NP], FP16)
        nc.gpsimd.memset(wthi_c[:, :, :, 77:], 0.0)
        wthi8_c = const.tile([128, 2, 8, NP], FP8)

        # --- input stream: head (weights + batch 0) ungated and issued up
        # front (measured near-optimal); batches 1-3 gated 1-back per chain
        # so they cannot round-robin-steal bandwidth from earlier tensors.
        def gate(dst_t, src_t, eng):
            eng.tensor_copy(dst_t[0:1, 0:1, 0:1], src_t[0:1, 0:1, 0:1])

        nc.sync.dma_start(wflo[:], wflo_d.rearrange("p (t x) -> p t x", t=2))

        states = {b: {"b": b} for b in range(BPC)}
        for b in range(BPC):
            states[b]["ft"] = big.tile([128, 8, 512], FP16, tag="ft", name="ft")
            states[b]["ft8"] = big.tile([128, 8, 512], FP8, tag="ft8", name="ft8")
        nc.sync.dma_start(
            states[0]["ft"][:], ftp16_d[0].rearrange("p (t x) -> p t x", t=8)
        )
        nc.sync.dma_start(
            states[0]["ft8"][:], ftp8_d[0].rearrange("p (t x) -> p t x", t=8)
        )
        for b in range(1, BPC):
            gate(states[b]["ft"], states[b - 1]["ft"], nc.gpsimd)
            nc.gpsimd.dma_start(
                states[b]["ft"][:], ftp16_d[b].rearrange("p (t x) -> p t x", t=8)
            )
            gate(states[b]["ft8"], states[b - 1]["ft8"], nc.gpsimd)
            nc.gpsimd.dma_start(
                states[b]["ft8"][:], ftp8_d[b].rearrange("p (t x) -> p t x", t=8)
            )

        def prep_a(st):
            # we0 = fp16(word_emb) @ W_fc^T (77, 1024): 2-chain, hi movings
            # first (the wfc-lo DMA lands later), 4 LDW + 8 MM
            wb = TAIL0 + st["b"] * NP
            ph = [
                mm_ps.tile([128, 512], FP32, tag="mm", name=f"we0p{h}")
                for h in range(2)
            ]
            n_in = {0: 0, 1: 0}
            for src in (wfcT, wflo):  # wfc hi cols, then lo cols
                for dc in range(2):
                    for h in range(2):
                        nc.tensor.matmul(
                            ph[h][:NP, :],
                            wfcT[:, dc, ds(wb, NP)],
                            src[:, dc, ds(h * 512, 512)],
                            start=(n_in[h] == 0),
                            stop=(n_in[h] == 3),
                        )
                        n_in[h] += 1
            we0hi = st["we0h"] = med.tile([128, 1024], FP16, tag="we0hi", name="we0hi")
            nc.vector.tensor_copy(we0hi[:NP, :512], ph[0][:NP, :])
            nc.vector.tensor_copy(we0hi[:NP, 512:], ph[1][:NP, :])
            return st

        def prep_b(st):
            # wt = we0hi transposed: 8 PE transposes -> psum fp16 -> sbuf;
            # wt8 = e5m2(wt * 2^-10) for the fp8 lo-correction matmuls
            par = st["b"] % 2
            wthi = st["wthi"] = wthi_c[:, par]
            wthi8 = st["wthi8"] = wthi8_c[:, par]
            pt = tp_ps.tile([128, 8, NP], FP16, tag="tp")
            for g in range(8):
                nc.tensor.matmul(
                    pt[:, g, :77],
                    st["we0h"][:77, ts(g, 128)],
                    identh[:77, :77],
                    is_transpose=True,
                    start=(g == 0),
                    stop=(g == 7),
                )
            nc.vector.tensor_copy(wthi[:, :, :77], pt[:, :, :77])
            nc.scalar.mul(wthi8[:], wthi[:], 1.0 / LOSCALE)
            return st

        def score(st):
            # S^T = wt^T @ FT_hi + wt8^T @ FT_lo8 (77, 512), one PSUM group
            sps = st["sps"] = sp_ps.tile([128, 512], FP32, tag="sp", name="sps")
            for kt in range(8):
                nc.tensor.matmul(
                    sps[:NP, :],
                    st["wthi"][:, kt, :],
                    st["ft"][:, kt, :],
                    start=(kt == 0),
                    stop=False,
                )
            if USE_DR:
                for t in range(4):
                    nc.tensor.matmul(
                        sps[:NP, :],
                        st["wthi8"][:, ds(2 * t, 2), :],
                        st["ft8"][:, ds(2 * t, 2), :],
                        start=False,
                        stop=(t == 3),
                        perf_mode=mybir.MatmulPerfMode.DoubleRow,
                    )
            else:
                for kt in range(8):
                    nc.tensor.matmul(
                        sps[:NP, :],
                        st["wthi8"][:, kt, :],
                        st["ft8"][:, kt, :],
                        start=False,
                        stop=(kt == 7),
                    )

        def soft_a(st):
            # E = exp(S - 96) via exp(0.5 S - 48)^2 (fixed shift: softmax-
            # exact, overflow-safe); eT in f32r for the PE sums matmul
            eh = med.tile([128, 512], FP32, tag="ehalf", name="ehalf")
            nc.scalar.activation(
                eh[:77, :], st["sps"][:77, :], AF.Exp, bias=ebias[:77, :], scale=0.5
            )
            eT = st["eT"] = med.tile([128, 512], F32R, tag="eT", name="eT")
            nc.vector.tensor_mul(eT[:77, :], eh[:77, :], eh[:77, :])

        def soft_b(st):
            # denominators: (1, C) row via f32r ones matmul; 1/row on DVE;
            # f32r re-round; K=1 f32r broadcast matmul; A = E/sums -> fp16
            eT = st["eT"]
            sus = su_ps.tile([128, 512], FP32, tag="sums", name="sums")
            nc.tensor.matmul(sus[:8, :], onesr[:77, :], eT[:77, :])
            rr = med.tile([128, 512], FP32, tag="rrow", name="rrow")
            nc.vector.reciprocal_approx_fast(rr[:1, :], sus[:1, :])
            rrr = med.tile([128, 512], F32R, tag="rrowr", name="rrowr")
            nc.vector.tensor_copy(rrr[:1, :], rr[:1, :])
            rb = mm_ps.tile([128, 512], FP32, tag="mm", name="rb")
            nc.tensor.matmul(rb[:NP, :], ones1r[:1, :NP], rrr[:1, :])
            at16 = st["at"] = med.tile([128, 512], FP16, tag="at", name="at")
            nc.vector.tensor_mul(at16[:77, :], eT[:77, :], rb[:77, :])

        def o_phase(st, b, act_only=False):
            # per c-tile: O = A-slice^T @ we0hi; fp16 out. act_only routes
            # both PSUM->SBUF copies to ACT so the epilogue's DVE chain
            # (recip/at16 of the last batch) is not queued behind them.
            at16, we0hi = st["at"], st["we0h"]
            for ct in range(4):
                po0 = mm_ps.tile([128, 512], FP32, tag="mm")
                nc.tensor.matmul(po0[:], at16[:77, ts(ct, 128)], we0hi[:77, :512])
                po1 = mm_ps.tile([128, 512], FP32, tag="mm")
                nc.tensor.matmul(po1[:], at16[:77, ts(ct, 128)], we0hi[:77, 512:1024])
                ob = outp.tile([128, 1024], FP16, tag="outb")
                nc.scalar.copy(ob[:, :512], po0[:])
                if act_only:
                    nc.scalar.copy(ob[:, 512:], po1[:])
                else:
                    nc.vector.tensor_copy(ob[:, 512:], po1[:])
                nc.sync.dma_start(out_d[b, ts(ct, 128), :], ob[:])

        # ---- software pipeline ----
        prep_a(states[0])
        for b in range(BPC):
            prep_b(states[b])
            if b > 0:
                soft_b(states[b - 1])
            if b + 1 < BPC:
                prep_a(states[b + 1])
            score(states[b])
            soft_a(states[b])
            if b > 0:
                o_phase(states[b - 1], b - 1, act_only=(b == BPC - 1))
        soft_b(states[BPC - 1])
        o_phase(states[BPC - 1], BPC - 1)


def _build():
    nc = bacc.Bacc(
        "TRN2",
        target_bir_lowering=False,
        debug=False,
        enable_asserts=False,
        num_devices=N_CORES,
    )
    ftp16_d = nc.declare_dram_parameter("ftp16", [BPC, 128, 8 * C], FP16, isOutput=False)
    ftp8_d = nc.declare_dram_parameter("ftp8", [BPC, 128, 8 * C], FP8, isOutput=False)
    wfcT_d = nc.declare_dram_parameter("wfcT", [128, 2 * WHI_W], FP16, isOutput=False)
    wflo_d = nc.declare_dram_parameter("wflo", [128, 2 * HW2], FP16, isOutput=False)
    out_d = nc.declare_dram_parameter("out", [BPC, C, HW2], FP16, isOutput=True)
    with tile.TileContext(nc) as tc:
        _body(nc, tc, ftp16_d, ftp8_d, wfcT_d, wflo_d, out_d)
    nc.finalize()
    return nc


_CACHE = {}


def kernel(feat, word_emb, W_fc, b_fc, **run_kwargs):
    global LAST_RESULT
    feat = np.asarray(feat, dtype=np.float32).reshape(B, C, HW2)
    word_emb = np.ascontiguousarray(np.asarray(word_emb, dtype=np.float32))
    W_fc = np.ascontiguousarray(np.asarray(W_fc, dtype=np.float32))
    b_fc = np.asarray(b_fc, dtype=np.float32)

    # feat -> (B, HW2, C): hi fp16 + lo as e5m2(lo * 2^10), both pre-swizzled
    # to the SBUF layout [128, kt, 512] so every DMA line is contiguous.
    featT = np.ascontiguousarray(feat.transpose(0, 2, 1))  # (B, HW2, C)
    fthi = featT.astype(np.float16)
    ftlo8 = ((featT - fthi.astype(np.float32)) * LOSCALE).astype(
        ml_dtypes.float8_e5m2
    )

    def swz(x):  # (B, HW2, C) -> (B, 128, 8*C) with [b, p, t*C+c] = x[b, t*128+p, c]
        return np.ascontiguousarray(
            x.reshape(B, 8, 128, C).transpose(0, 2, 1, 3).reshape(B, 128, 8 * C)
        )

    fthi_s = swz(fthi)
    ftlo8_s = swz(ftlo8)

    # word_emb^T hi -> (B, WORD_DIM, NP) fp16, cols 77:80 = 0
    wembT = np.ascontiguousarray(word_emb.transpose(0, 2, 1))  # (B, 256, 77)
    wembTp = np.zeros((B, WORD_DIM, NP), dtype=np.float16)
    wembTp[:, :, :77] = wembT.astype(np.float16)

    # W_fc^T: hi rows [wfc_hi(1024) | wembT_hi(4*80)], lo separate; both
    # pre-swizzled to [128, dc, x]
    wfcT = np.ascontiguousarray(W_fc.T)  # (256, 1024)
    wfhi = wfcT.astype(np.float16)
    wflo = (wfcT - wfhi.astype(np.float32)).astype(np.float16)
    wfcTp = np.zeros((N_CORES, WORD_DIM, WHI_W), dtype=np.float16)
    wfcTp[:, :, :HW2] = wfhi
    for i in range(N_CORES):
        for b in range(BPC):
            wfcTp[i, :, TAIL0 + b * NP : TAIL0 + (b + 1) * NP] = wembTp[i * BPC + b]
    wfcTp_s = np.ascontiguousarray(
        wfcTp.reshape(N_CORES, 2, 128, WHI_W).transpose(0, 2, 1, 3)
    ).reshape(N_CORES, 128, 2 * WHI_W)
    wflo_s = np.ascontiguousarray(
        wflo.reshape(2, 128, HW2).transpose(1, 0, 2)
    ).reshape(128, 2 * HW2)

    if "nc" not in _CACHE:
        _CACHE["nc"] = _build()
    nc = _CACHE["nc"]

    in_maps = [
        {
            "ftp16": fthi_s[i * BPC : (i + 1) * BPC],
            "ftp8": ftlo8_s[i * BPC : (i + 1) * BPC],
            "wfcT": wfcTp_s[i],
            "wflo": wflo_s,
        }
        for i in range(N_CORES)
    ]
    res = run_bass_kernel_spmd(nc, in_maps, list(range(N_CORES)), **run_kwargs)
    LAST_RESULT = res
    out16 = np.concatenate([res.results[i]["out"] for i in range(N_CORES)], axis=0)
    # b_fc shifts all logits of a softmax row equally (no effect on A) and
    # adds linearly to the output: out = A @ we0 + b_fc. Exact identity.
    out = out16.astype(np.float32) + b_fc.reshape(1, 1, HW2)
    return out.reshape(B, C, H, W).astype(np.float32)


# revision 15
# speedup vs baseline: 1.3569x; 1.0892x over previous
"""Channel-attention kernel for Trainium2 (8 NeuronCores, batch-parallel).

Reference computation per batch b (feat (C, HW2), word_emb (N, D)):
    we0   = word_emb @ W_fc^T                 (N, HW2)
    S     = feat @ we0^T                      (C, N)   [b_fc shifts every logit
                                                        of a row equally -> the
                                                        softmax is invariant]
    A     = softmax(S, axis=-1)
    out   = A @ we0 + b_fc                    (C, HW2) [b_fc added on host]

Precision scheme: fp16 2-chain with an fp8 lo-correction for feat. The
harness gate is scale-relative absmax (~2e-2); this scheme measures 9.3e-3
on HW. Structure:
    S ~= fp16(we0) @ feat_hi  +  [fp16(we0)*2^-10]_e5m2 @ [feat_lo*2^10]_e5m2
The fp8 product scales cancel exactly, so the correction accumulates into
the same PSUM group; feat_lo at fp8 cuts its HBM traffic in half.

Device dataflow per batch (one NeuronCore handles B/8 = 4 batches):
    we0hi    = fp16(wembT_hi^T @ (wfc_hi|wfc_lo))   (PE, 4 LDW + 8 MM F=512)
    wt       = PE transposes of we0hi               (8 transposes)
    wt8      = e5m2(wt * 2^-10)                     (ACT scaled copy)
    S^T      = wt^T@FT_hi + wt8^T@FT_lo8            (PE, 8 fp16 MM + 4
                                                     DoubleRow fp8 MM F=512)
    eT       = exp(0.5 S^T - 48)^2 as f32r          (ACT exp + DVE square)
    sums     = ones^T @ eT                          (PE f32r, 1 MM F=512)
    rb       = bcast 1/sums via K=1 f32r MM         (PE, 1 MM F=512)
    at16     = fp16(eT * rb)                        (DVE)
    O        = at16-chunk^T @ we0hi                 (PE, 8 MM F=512), fp16 out

DMA plumbing (from trace analysis):
  - The 16 DMA engines service outstanding descriptors ROUND-ROBIN, so
    issue order does not give delivery order: every concurrently-queued
    descriptor steals bandwidth from the one you need first. Inputs are
    therefore gated into an explicitly SEQUENTIAL chain with 1-element
    "corner copy" dependencies on the (otherwise idle) GpSimd queue:
    wfcT -> wflo -> ft16(0) -> ft16(1) -> ... and a parallel fp8 chain
    ft8(0) -> ft8(1) -> ... headed by one DVE cast off ft16(0).
  - All tensors are host-pre-swizzled so each DMA is one descriptor with
    2-8KB contiguous per-partition lines (512B fp8 lines measured ~18%
    slower aggregate).
  - Output DMAs stay on the Sync queue and trail the input chain.

Compute scheduling: slot b emission = prep_b(b) | soft_b(b-1) | prep_a(b+1)
| score(b) | soft_a(b) | o(b-1).  Queue-order invariants: exp(b) lands on
the ACT queue BEFORE o(b-1)'s PSUM->SBUF copies, recip(b-1) lands on the
DVE queue before o(b-1)'s copies (strict-FIFO queues otherwise head-of-line
block the softmax chain behind output copies, idling the PE until the HAM
clock-gate re-throttles it to 1.2 GHz).  o(BPC-2)'s copies go ACT-only so
the epilogue's recip/at16 are not queued behind them on the DVE.
"""

import numpy as np
import ml_dtypes

import concourse.bass as bass
import concourse.mybir as mybir
import concourse.tile as tile
from concourse import bacc
from concourse.bass import ds, ts
from concourse.bass_utils import run_bass_kernel_spmd
from concourse.masks import make_identity

B, C, HW2 = 32, 512, 1024
N_WORDS, WORD_DIM = 77, 256
H = W = 32
N_CORES = 8
BPC = B // N_CORES  # batches per core
NP = 80  # n padded to even (f32r/ISA friendliness + zero-padded stationaries)

FP32 = mybir.dt.float32
FP16 = mybir.dt.float16
FP8 = mybir.dt.float8e5
F32R = mybir.dt.float32r
AF = mybir.ActivationFunctionType

TAIL0 = HW2  # wembT-hi columns start within wfcT (per dc)
WHI_W = HW2 + BPC * NP  # per-dc row of wfcT: [wfc_hi | wembT_hi*4]
LOSCALE = 2.0**10
USE_DR = True  # DoubleRow fp8 for the score lo-correction (4 MMs vs 8)

LAST_RESULT = None  # BassKernelResults of the most recent run (for test.py)


def _body(nc, tc, ftp16_d, ftp8_d, wfcT_d, wflo_d, out_d):
    from contextlib import ExitStack

    with ExitStack() as ctx:
        const = ctx.enter_context(tc.tile_pool(name="const", bufs=1))
        big = ctx.enter_context(tc.tile_pool(name="big", bufs=4))
        med = ctx.enter_context(tc.tile_pool(name="med", bufs=3))
        wep = ctx.enter_context(tc.tile_pool(name="wep", bufs=4))
        outp = ctx.enter_context(tc.tile_pool(name="outp", bufs=8))
        mm_ps = ctx.enter_context(tc.tile_pool(name="mm_ps", bufs=4, space="PSUM"))
        sp_ps = ctx.enter_context(tc.tile_pool(name="sp_ps", bufs=2, space="PSUM"))
        tp_ps = ctx.enter_context(tc.tile_pool(name="tp_ps", bufs=1, space="PSUM"))
        su_ps = ctx.enter_context(tc.tile_pool(name="su_ps", bufs=1, space="PSUM"))

        wfcT = const.tile([128, 2, WHI_W], FP16)
        nc.sync.dma_start(wfcT[:], wfcT_d.rearrange("p (t x) -> p t x", t=2))
        wflo = const.tile([128, 2, HW2], FP16)

        # constants (also fills the GpSimd queue before the DMA gate chain)
        identh = const.tile([128, 128], FP16)
        ident = const.tile([128, 128], FP32)
        make_identity(nc, ident[:])
        nc.vector.tensor_copy(identh[:], ident[:])
        ones_f = const.tile([128, 8], FP32)
        nc.gpsimd.memset(ones_f[:], 1.0)
        onesr = const.tile([128, 8], F32R)
        nc.vector.tensor_copy(onesr[:], ones_f[:])
        ones1f = const.tile([128, 128], FP32)
        nc.gpsimd.memset(ones1f[:], 1.0)
        ones1r = const.tile([128, 128], F32R)
        nc.vector.tensor_copy(ones1r[:], ones1f[:])
        ebias = const.tile([128, 1], FP32)
        nc.gpsimd.memset(ebias[:], -48.0)
        gatescr = const.tile([128, 8], FP32)
        # wt double-buffer lives in const so its pad columns are zeroed once
        wthi_c = const.tile([128, 2, 8, NP], FP16)
        nc.gpsimd.memset(wthi_c[:, :, :, 77:], 0.0)
        wthi8_c = const.tile([128, 2, 8, NP], FP8)

        # --- input stream: head (weights + batch 0) ungated and issued up
        # front (measured near-optimal); batches 1-3 gated 1-back per chain
        # so they cannot round-robin-steal bandwidth from earlier tensors.
        def gate(dst_t, src_t, eng):
            eng.tensor_copy(dst_t[0:1, 0:1, 0:1], src_t[0:1, 0:1, 0:1])

        nc.sync.dma_start(wflo[:], wflo_d.rearrange("p (t x) -> p t x", t=2))

        states = {b: {"b": b} for b in range(BPC)}
        for b in range(BPC):
            states[b]["ft"] = big.tile([128, 8, 512], FP16, tag="ft", name="ft")
            states[b]["ft8"] = big.tile([128, 8, 512], FP8, tag="ft8", name="ft8")
        nc.sync.dma_start(
            states[0]["ft"][:], ftp16_d[0].rearrange("p (t x) -> p t x", t=8)
        )
        nc.sync.dma_start(
            states[0]["ft8"][:], ftp8_d[0].rearrange("p (t x) -> p t x", t=8)
        )
        for b in range(1, BPC):
            gate(states[b]["ft"], states[b - 1]["ft"], nc.gpsimd)
            nc.gpsimd.dma_start(
                states[b]["ft"][:], ftp16_d[b].rearrange("p (t x) -> p t x", t=8)
            )
            gate(states[b]["ft8"], states[b - 1]["ft8"], nc.gpsimd)
            nc.gpsimd.dma_start(
                states[b]["ft8"][:], ftp8_d[b].rearrange("p (t x) -> p t x", t=8)
            )

        def prep_a(st):
            # we0 = fp16(word_emb) @ W_fc^T (77, 1024): 2-chain, hi movings
            # first (the wfc-lo DMA lands later), 4 LDW + 8 MM
            wb = TAIL0 + st["b"] * NP
            ph = [
                mm_ps.tile([128, 512], FP32, tag="mm", name=f"we0p{h}")
                for h in range(2)
            ]
            n_in = {0: 0, 1: 0}
            for src in (wfcT, wflo):  # wfc hi cols, then lo cols
                for dc in range(2):
                    for h in range(2):
                        nc.tensor.matmul(
                            ph[h][:NP, :],
                            wfcT[:, dc, ds(wb, NP)],
                            src[:, dc, ds(h * 512, 512)],
                            start=(n_in[h] == 0),
                            stop=(n_in[h] == 3),
                        )
                        n_in[h] += 1
            we0hi = st["we0h"] = wep.tile([128, 1024], FP16, tag="we0hi", name="we0hi")
            nc.vector.tensor_copy(we0hi[:NP, :512], ph[0][:NP, :])
            nc.vector.tensor_copy(we0hi[:NP, 512:], ph[1][:NP, :])
            return st

        def prep_b(st):
            # wt = we0hi transposed: 8 PE transposes -> psum fp16 -> sbuf;
            # wt8 = e5m2(wt * 2^-10) for the fp8 lo-correction matmuls
            par = st["b"] % 2
            wthi = st["wthi"] = wthi_c[:, par]
            wthi8 = st["wthi8"] = wthi8_c[:, par]
            pt = tp_ps.tile([128, 8, NP], FP16, tag="tp")
            for g in range(8):
                nc.tensor.matmul(
                    pt[:, g, :77],
                    st["we0h"][:77, ts(g, 128)],
                    identh[:77, :77],
                    is_transpose=True,
                    start=(g == 0),
                    stop=(g == 7),
                )
            nc.vector.tensor_copy(wthi[:, :, :77], pt[:, :, :77])
            nc.scalar.mul(wthi8[:], wthi[:], 1.0 / LOSCALE)
            return st

        def score(st):
            # S^T = wt^T @ FT_hi + wt8^T @ FT_lo8 (77, 512), one PSUM group
            sps = st["sps"] = sp_ps.tile([128, 512], FP32, tag="sp", name="sps")
            for kt in range(8):
                nc.tensor.matmul(
                    sps[:NP, :],
                    st["wthi"][:, kt, :],
                    st["ft"][:, kt, :],
                    start=(kt == 0),
                    stop=False,
                )
            if USE_DR:
                for t in range(4):
                    nc.tensor.matmul(
                        sps[:NP, :],
                        st["wthi8"][:, ds(2 * t, 2), :],
                        st["ft8"][:, ds(2 * t, 2), :],
                        start=False,
                        stop=(t == 3),
                        perf_mode=mybir.MatmulPerfMode.DoubleRow,
                    )
            else:
                for kt in range(8):
                    nc.tensor.matmul(
                        sps[:NP, :],
                        st["wthi8"][:, kt, :],
                        st["ft8"][:, kt, :],
                        start=False,
                        stop=(kt == 7),
                    )

        def soft_a(st):
            # E = exp(S - 96) via exp(0.5 S - 48)^2 (fixed shift: softmax-
            # exact, overflow-safe); eT in f32r for the PE sums matmul
            eh = med.tile([128, 512], FP32, tag="ehalf", name="ehalf")
            nc.scalar.activation(
                eh[:77, :], st["sps"][:77, :], AF.Exp, bias=ebias[:77, :], scale=0.5
            )
            eT = st["eT"] = med.tile([128, 512], F32R, tag="eT", name="eT")
            nc.vector.tensor_mul(eT[:77, :], eh[:77, :], eh[:77, :])

        def soft_b1(st):
            # denominators: (1, C) row via f32r ones matmul; 1/row on DVE
            sus = st["sus"] = su_ps.tile([128, 512], FP32, tag="sums", name="sums")
            nc.tensor.matmul(sus[:8, :], onesr[:77, :], st["eT"][:77, :])
            rr = med.tile([128, 512], FP32, tag="rrow", name="rrow")
            nc.vector.reciprocal_approx_fast(rr[:1, :], sus[:1, :])
            rrr = st["rrr"] = med.tile([128, 512], F32R, tag="rrowr", name="rrowr")
            nc.vector.tensor_copy(rrr[:1, :], rr[:1, :])

        def soft_b2(st):
            # K=1 f32r broadcast matmul; A = E/sums -> fp16
            rb = mm_ps.tile([128, 512], FP32, tag="mm", name="rb")
            nc.tensor.matmul(rb[:NP, :], ones1r[:1, :NP], st["rrr"][:1, :])
            at16 = st["at"] = med.tile([128, 512], FP16, tag="at", name="at")
            nc.vector.tensor_mul(at16[:77, :], st["eT"][:77, :], rb[:77, :])

        def o_phase(st, b):
            # per c-tile: O = A-slice^T @ we0hi; fp16 out via ACT+DVE copies.
            # Output DMAs for the first two batches are queue-position-gated
            # on the gpsimd queue behind a scratch read of a later input
            # tile, so they cannot round-robin-steal DMA bandwidth from the
            # input stream the compute still depends on.
            at16, we0hi = st["at"], st["we0h"]
            gate_src = states[b + 2]["ft"] if b + 2 < BPC else None
            if gate_src is not None:
                nc.gpsimd.tensor_copy(gatescr[0:1, 0:1], gate_src[0:1, 0:1, 0:1])
            for ct in range(4):
                po0 = mm_ps.tile([128, 512], FP32, tag="mm")
                nc.tensor.matmul(po0[:], at16[:77, ts(ct, 128)], we0hi[:77, :512])
                po1 = mm_ps.tile([128, 512], FP32, tag="mm")
                nc.tensor.matmul(po1[:], at16[:77, ts(ct, 128)], we0hi[:77, 512:1024])
                ob = outp.tile([128, 1024], FP16, tag="outb")
                nc.scalar.copy(ob[:, :512], po0[:])
                nc.vector.tensor_copy(ob[:, 512:], po1[:])
                eng = nc.gpsimd if gate_src is not None else nc.sync
                eng.dma_start(out_d[b, ts(ct, 128), :], ob[:])

        # ---- software pipeline ----
        # Slot b: soft_b1(b-1) | prep_a(b+2) | soft_b2(b-1) | score(b) |
        # soft_a(b) | prep_b(b+1) | o(b-1).  PE order: sums, prep_a MMs
        # (cover recip), rb, score, tp (cover exp/square), o.  exp(b) lands
        # on ACT before o(b-1)'s copies; recip(b-1) on DVE before them.
        prep_a(states[0])
        prep_b(states[0])
        prep_a(states[1])
        for b in range(BPC):
            if b > 0:
                soft_b1(states[b - 1])
            if b + 2 < BPC:
                prep_a(states[b + 2])
            if b > 0:
                soft_b2(states[b - 1])
            score(states[b])
            soft_a(states[b])
            if b + 1 < BPC:
                prep_b(states[b + 1])
            if b > 0:
                o_phase(states[b - 1], b - 1)
        soft_b1(states[BPC - 1])
        soft_b2(states[BPC - 1])
        o_phase(states[BPC - 1], BPC - 1)


def _build():
    nc = bacc.Bacc(
        "TRN2",
        target_bir_lowering=False,
        debug=False,
        enable_asserts=False,
        num_devices=N_CORES,
    )
    ftp16_d = nc.declare_dram_parameter("ftp16", [BPC, 128, 8 * C], FP16, isOutput=False)
    ftp8_d = nc.declare_dram_parameter("ftp8", [BPC, 128, 8 * C], FP8, isOutput=False)
    wfcT_d = nc.declare_dram_parameter("wfcT", [128, 2 * WHI_W], FP16, isOutput=False)
    wflo_d = nc.declare_dram_parameter("wflo", [128, 2 * HW2], FP16, isOutput=False)
    out_d = nc.declare_dram_parameter("out", [BPC, C, HW2], FP16, isOutput=True)
    with tile.TileContext(nc) as tc:
        _body(nc, tc, ftp16_d, ftp8_d, wfcT_d, wflo_d, out_d)
    nc.finalize()
    return nc


_CACHE = {}


def kernel(feat, word_emb, W_fc, b_fc, **run_kwargs):
    global LAST_RESULT
    feat = np.asarray(feat, dtype=np.float32).reshape(B, C, HW2)
    word_emb = np.ascontiguousarray(np.asarray(word_emb, dtype=np.float32))
    W_fc = np.ascontiguousarray(np.asarray(W_fc, dtype=np.float32))
    b_fc = np.asarray(b_fc, dtype=np.float32)

    # feat -> (B, HW2, C): hi fp16 + lo as e5m2(lo * 2^10), both pre-swizzled
    # to the SBUF layout [128, kt, 512] so every DMA line is contiguous.
    featT = np.ascontiguousarray(feat.transpose(0, 2, 1))  # (B, HW2, C)
    fthi = featT.astype(np.float16)
    ftlo8 = ((featT - fthi.astype(np.float32)) * LOSCALE).astype(
        ml_dtypes.float8_e5m2
    )

    def swz(x):  # (B, HW2, C) -> (B, 128, 8*C) with [b, p, t*C+c] = x[b, t*128+p, c]
        return np.ascontiguousarray(
            x.reshape(B, 8, 128, C).transpose(0, 2, 1, 3).reshape(B, 128, 8 * C)
        )

    fthi_s = swz(fthi)
    ftlo8_s = swz(ftlo8)

    # word_emb^T hi -> (B, WORD_DIM, NP) fp16, cols 77:80 = 0
    wembT = np.ascontiguousarray(word_emb.transpose(0, 2, 1))  # (B, 256, 77)
    wembTp = np.zeros((B, WORD_DIM, NP), dtype=np.float16)
    wembTp[:, :, :77] = wembT.astype(np.float16)

    # W_fc^T: hi rows [wfc_hi(1024) | wembT_hi(4*80)], lo separate; both
    # pre-swizzled to [128, dc, x]
    wfcT = np.ascontiguousarray(W_fc.T)  # (256, 1024)
    wfhi = wfcT.astype(np.float16)
    wflo = (wfcT - wfhi.astype(np.float32)).astype(np.float16)
    wfcTp = np.zeros((N_CORES, WORD_DIM, WHI_W), dtype=np.float16)
    wfcTp[:, :, :HW2] = wfhi
    for i in range(N_CORES):
        for b in range(BPC):
            wfcTp[i, :, TAIL0 + b * NP : TAIL0 + (b + 1) * NP] = wembTp[i * BPC + b]
    wfcTp_s = np.ascontiguousarray(
        wfcTp.reshape(N_CORES, 2, 128, WHI_W).transpose(0, 2, 1, 3)
    ).reshape(N_CORES, 128, 2 * WHI_W)
    wflo_s = np.ascontiguousarray(
        wflo.reshape(2, 128, HW2).transpose(1, 0, 2)
    ).reshape(128, 2 * HW2)

    if "nc" not in _CACHE:
        _CACHE["nc"] = _build()
    nc = _CACHE["nc"]

    in_maps = [
        {
            "ftp16": fthi_s[i * BPC : (i + 1) * BPC],
            "ftp8": ftlo8_s[i * BPC : (i + 1) * BPC],
            "wfcT": wfcTp_s[i],
            "wflo": wflo_s,
        }
        for i in range(N_CORES)
    ]
    res = run_bass_kernel_spmd(nc, in_maps, list(range(N_CORES)), **run_kwargs)
    LAST_RESULT = res
    out16 = np.concatenate([res.results[i]["out"] for i in range(N_CORES)], axis=0)
    # b_fc shifts all logits of a softmax row equally (no effect on A) and
    # adds linearly to the output: out = A @ we0 + b_fc. Exact identity.
    out = out16.astype(np.float32) + b_fc.reshape(1, 1, HW2)
    return out.reshape(B, C, H, W).astype(np.float32)
